# revision 20
# baseline (speedup 1.0000x reference)
"""Trainium2 Bass kernel for nn_MilliesRNN (B=256, T=512, L=128, O=64, H=576).

Strategy (v2 — weight-load-bound optimization):
- Data-parallel over batch: 8 cores x 32 sequences; weights replicated;
  states SBUF-resident, transposed ([hidden-chunk, batch]).
- The per-step PE cost is LDWEIGHTS-bound (stationary-tile columns stream
  at ~1 col/cycle; moving N=32 only). v2 cuts weight-load cost by:
  * fp8 (TRN float8e3 = e3m4) stationary tiles for all large weights, with
    bf16 moving states (mixed-dtype matmul, verified bit-exact on HW) ->
    4x faster weight load via FWL. Global power-of-2 scale per output
    group (S_zv, S_p, S_zm); unscale via activation `scale` immediate.
  * Mv (=thal[:, :64] @ h2o_w) merged into W1's m4 column-tiles: the same
    pass over hv computes zv-tail AND p. Mo (=h2od_w^T) merged into W2's
    m4 tiles (bf16 for output accuracy): same pass computes zm-tail AND om.
  * All stationary tiles zero-padded to K=128 (uniform full-row moving
    blocks; ones-row at row 65 carries biases, hold row at 64).
  * Structural 1-step software pipeline: at iter s the merged A-tiles
    compute [zv(s+1) | p(s)] from hv(s); B-tiles compute [zm(s+1) | om(s)]
    from hm(s); wtm joins zm(s) late (after ot(s) exists).
- Validated numerics (numpy bit-approximate sim, full batch): 1.24e-2
  rel L2 (tolerance 2e-2); all-bf16 variant: 3.2e-3.
"""

import numpy as np
import ml_dtypes

import concourse.bass as bass
import concourse.mybir as mybir
import concourse.tile as tile
from concourse.bass_utils import run_bass_kernel_spmd
from concourse.vector_clock import ScopedClock

# ----------------------------------------------------------------------------
# Workarounds: this walrus build only supports ONE sync-wait per instruction.
# ----------------------------------------------------------------------------
_MAXW = 1


def _patched_drain_and_barrier(self, tick_clock, wait_clock):
    nc = self.nc
    drain_inst = nc.sync.drain()
    wait_clock.add_sem_waits(
        drain_inst.ins, ScopedClock({None: tick_clock.global_clock})
    )
    waits = list(drain_inst.ins.sync_info.on_wait)
    if len(waits) > _MAXW:
        drain_inst.ins.sync_info = mybir.SyncInfo(
            on_wait=waits[:_MAXW], on_update=[]
        )
        rest = waits[_MAXW:]
        for i in range(0, len(rest), _MAXW):
            nop = nc.sync.nop(nofuse=True)
            nop.ins.sync_info = mybir.SyncInfo(
                on_wait=rest[i : i + _MAXW], on_update=[]
            )
    nc.all_engine_barrier()
    assert self.sems is not None
    popped = nc._tile_sem_poison_stack.pop()
    assert popped is self._sem_poison
    nc.clear_and_free_semaphores(list(self.sems.allocated().values()))
    nc.all_engine_barrier()


tile.TileContext._drain_and_barrier = _patched_drain_and_barrier

_wfix_ctr = [0]


def _split_waits(nc, maxw=_MAXW):
    """Move excess sync-waits onto preceding same-engine nops."""
    n_split = 0
    for f in nc.m.functions:
        for b in f.blocks:
            lst = b.instructions
            i = 0
            while i < len(lst):
                inst = lst[i]
                si = getattr(inst, "sync_info", None)
                if si is not None:
                    waits = list(si.on_wait)
                    if len(waits) > maxw:
                        n_split += 1
                        inst.sync_info = mybir.SyncInfo(
                            on_wait=waits[:maxw], on_update=list(si.on_update)
                        )
                        rest = waits[maxw:]
                        for j in range(0, len(rest), maxw):
                            nop = mybir.InstNoOp(
                                name=f"WFIX-{_wfix_ctr[0]}", ins=[], outs=[]
                            )
                            _wfix_ctr[0] += 1
                            nop.engine = inst.engine
                            nop.sync_info = mybir.SyncInfo(
                                on_wait=rest[j : j + maxw], on_update=[]
                            )
                            lst.insert(i, nop)
                            i += 1
                i += 1
    return n_split


def _thin_updates(nc, engines=("EngineType.PE",)):
    """Drop engine-clock sem-incs at positions nobody waits on, and renumber
    the remaining wait thresholds to ranks within the kept set."""
    thresholds = {}
    inc_engines = {}
    _bb_incs = {}
    for f in nc.m.functions:
        for b in f.blocks:
            for inst in b.instructions:
                si = getattr(inst, "sync_info", None)
                if si is None:
                    continue
                for w in si.on_wait:
                    if w.wait_value is not None and w.ant_name:
                        thresholds.setdefault(w.ant_name, set()).add(
                            w.wait_value
                        )
                for u in si.on_update:
                    if u.update_mode == "sem-inc" and u.ant_name:
                        inc_engines.setdefault(u.ant_name, set()).add(
                            str(inst.engine)
                        )
                        k = (u.ant_name, id(b))
                        _bb_incs[k] = _bb_incs.get(k, 0) + 1
    eligible = {
        s for s, engs in inc_engines.items()
        if engs == set(engines) and s in thresholds
    }
    main_bb = {}
    for (s, bid), n in _bb_incs.items():
        if s in eligible and n > main_bb.get(s, (None, 0))[1]:
            main_bb[s] = (bid, n)
    remap = {}
    n_removed = 0
    for f in nc.m.functions:
        for b in f.blocks:
            cum = {}
            kept = {}
            for inst in b.instructions:
                si = getattr(inst, "sync_info", None)
                if si is None or str(inst.engine) not in engines:
                    continue
                keep = []
                changed = False
                for u in si.on_update:
                    if (
                        u.update_mode == "sem-inc"
                        and u.update_value == 1
                        and u.ant_name in eligible
                        and not u.update_reg
                        and main_bb[u.ant_name][0] == id(b)
                    ):
                        s = u.ant_name
                        cum[s] = cum.get(s, 0) + 1
                        if cum[s] in thresholds[s]:
                            kept[s] = kept.get(s, 0) + 1
                            remap.setdefault(s, {})[cum[s]] = kept[s]
                            keep.append(u)
                        else:
                            n_removed += 1
                            changed = True
                    else:
                        keep.append(u)
                if changed:
                    inst.sync_info = mybir.SyncInfo(
                        on_wait=list(si.on_wait), on_update=keep
                    )
    for f in nc.m.functions:
        for b in f.blocks:
            for inst in b.instructions:
                si = getattr(inst, "sync_info", None)
                if si is None:
                    continue
                for w in si.on_wait:
                    m = remap.get(w.ant_name)
                    if m and w.wait_value in m:
                        w.wait_value = m[w.wait_value]
    return n_removed


# ----------------------------------------------------------------------------
# Problem constants
# ----------------------------------------------------------------------------
B, T, L, O, H = 256, 512, 128, 64, 576
N_CORES = 8
BC = B // N_CORES  # 32 batch per core
F32 = mybir.dt.float32
BF16 = mybir.dt.bfloat16
FP8 = mybir.dt.float8e3
AF = mybir.ActivationFunctionType

# megatile column-block indices (each [128, BC])
HV0 = 0      # hv chunks 0..3 at blocks 0..3
HV4 = 4      # hv-tail(0:64) + hold(s) row64 + ones row65
HM4 = 5      # hm-tail(0:64) + hold(s+1) row64 + ones row65
HM0 = 6      # hm chunks 0..3 at blocks 6..9
OTB = 10     # zeros top, ot at rows 64:128
OMB = 11     # zeros top, om at rows 64:128
NST = 12

OUT_RING = 16  # output staging ring blocks (DMA every OUT_RING//2)


def build_nc(T_steps=T, loop_niter=None, debug=False):
    """Per-core SPMD bass kernel, fully unrolled over T_steps.

    loop_niter: timing-only mode — wraps the body in a For_i hardware loop
    (outputs garbage past the first pass; per-step time via wall deltas)."""
    nc = bass.Bass()
    TB = T_steps * BC

    if debug:
        d_dbg = nc.dram_tensor("DBG", [128, NST * BC], BF16,
                               kind="ExternalOutput")
    d_xT = nc.dram_tensor("xT", [L, TB], BF16, kind="ExternalInput")
    d_hd = nc.dram_tensor("HD", [1, TB], BF16, kind="ExternalInput")
    d_wa = nc.dram_tensor("WA", [128, 25 * 128], FP8, kind="ExternalInput")
    d_xi = nc.dram_tensor("XI", [128, 4 * 128], FP8, kind="ExternalInput")
    d_wb = nc.dram_tensor("WB", [128, 20 * 128], FP8, kind="ExternalInput")
    d_wb4 = nc.dram_tensor("WB4", [128, 5 * 128], BF16, kind="ExternalInput")
    d_wt = nc.dram_tensor("WT", [128, 4 * 128], FP8, kind="ExternalInput")
    d_wt4 = nc.dram_tensor("WT4", [128, 64], BF16, kind="ExternalInput")
    d_th = nc.dram_tensor("TH", [128, 64], FP8, kind="ExternalInput")
    d_id = nc.dram_tensor("ID64", [128, 64], BF16, kind="ExternalInput")
    d_st = nc.dram_tensor("ST0", [128, NST * BC], BF16, kind="ExternalInput")
    d_out = nc.dram_tensor("OUT", [O, TB], F32, kind="ExternalOutput")

    frees = []
    with tile.TileContext(nc) as tc:

        def mk(name, shape, dtype):
            t, fr = tc.tile(shape, dtype, name=name)
            frees.append(fr)
            return t

        xTs = mk("xTs", [L, TB], BF16)
        wa = mk("wa", [128, 25 * 128], FP8)
        xi = mk("xi", [128, 4 * 128], FP8)
        wb = mk("wb", [128, 20 * 128], FP8)
        wb4 = mk("wb4", [128, 5 * 128], BF16)
        wt = mk("wt", [128, 4 * 128], FP8)
        wt4 = mk("wt4", [128, 64], BF16)
        th = mk("th", [128, 64], FP8)
        id64 = mk("id64", [128, 64], BF16)
        st = mk("st", [128, NST * BC], BF16)
        outs = mk("outs", [128, OUT_RING * BC], F32)
        # hold row staged at partition 64 (DVE cannot read partition 127,
        # and partition bases must match between copy src/dst)
        hrow = mk("hrow", [65, TB], BF16)

        # --- loads (xT split for early start) ---
        NXCH = 8
        xch = TB // NXCH
        for i in range(NXCH):
            nc.sync.dma_start(
                out=xTs[:, i * xch : (i + 1) * xch],
                in_=d_xT[:, i * xch : (i + 1) * xch],
            )
        for tl, dr in ((wa, d_wa), (xi, d_xi), (wb, d_wb), (wb4, d_wb4),
                       (wt, d_wt), (wt4, d_wt4), (th, d_th), (id64, d_id),
                       (st, d_st)):
            nc.sync.dma_start(out=tl, in_=dr[:, :])
        nc.sync.dma_start(out=hrow[64:65, :], in_=d_hd[:, :])

        _pp_cm = tc.tile_pool(name="psum", bufs=2, space="PSUM")
        pp = _pp_cm.__enter__()

        _started = {}

        def mm(out_ap, lhsT, rhs, start, stop):
            nc.tensor.matmul(
                out_ap, lhsT, rhs, start=start, stop=stop,
                skip_group_check=True,
            )

        def mmt(tl, out_ap, lhsT, rhs, stop=False):
            """Matmul with start=True exactly on the first MM into tile tl
            (start clears the whole PSUM bank's has_written bits)."""
            key = id(tl)
            started = _started.get(key, False)
            mm(out_ap, lhsT, rhs, not started, stop)
            _started[key] = True

        def blk(b):
            return st[:, b * BC : (b + 1) * BC]

        # scales are immediates baked at build time via SC? immediates must
        # be python floats at build time -> passed via module-level closure.
        inv_zv = build_nc._inv_zv
        inv_p = build_nc._inv_p
        inv_zm = build_nc._inv_zm

        state = {}

        def a_m4(s, zvp):
            """A(s) m4-part: [zv(s+1)-tail | p(s)] <- 5 wa-k-m4 + thm."""
            for k in range(5):
                w = wa[:, (k * 5 + 4) * 128 : (k * 5 + 4) * 128 + 128]
                r = blk(k) if k < 4 else blk(HV4)
                mmt(zvp, zvp[:, 4 * BC : 5 * BC], w, r)
            mmt(zvp, zvp[64:128, 4 * BC : 5 * BC], th[:, 0:64], blk(OMB))

        def hold_copy(s):
            """hv4h.row64 & hm4h.row64 <- hold(s+1) = x[127, (s+1)blk]."""
            src = hrow[64:65, (s + 1) * BC : (s + 2) * BC]
            nc.vector.tensor_copy(st[64:65, HV4 * BC : HV4 * BC + BC], src)
            nc.vector.tensor_copy(st[64:65, HM4 * BC : HM4 * BC + BC], src)

        def act_p(s, zvp):
            nc.scalar.activation(st[64:128, OTB * BC : OTB * BC + BC],
                                 zvp[64:128, 4 * BC : 5 * BC], AF.Tanh,
                                 bias=0.0, scale=inv_p)
            nc.vector.tensor_scalar_max(
                st[64:128, OTB * BC : OTB * BC + BC],
                st[64:128, OTB * BC : OTB * BC + BC], 0.0)

        def a_m03(s, zvp, krange, with_xi_id):
            for k in krange:
                for m in range(4):
                    w = wa[:, (k * 5 + m) * 128 : (k * 5 + m) * 128 + 128]
                    r = blk(k) if k < 4 else blk(HV4)
                    mmt(zvp, zvp[:, m * BC : (m + 1) * BC], w, r)
            if with_xi_id:
                xr = xTs[:, (s + 1) * BC : (s + 2) * BC]
                for m in range(4):
                    mmt(zvp, zvp[:, m * BC : (m + 1) * BC],
                        xi[:, m * 128 : (m + 1) * 128], xr)
                mmt(zvp, zvp[0:64, 4 * BC : 5 * BC], id64[:, 0:64],
                    blk(OTB), stop=True)

        def b2(s, zmp):
            """wtm x ot(s+1) -> completes zm(s+1)."""
            for m in range(4):
                mmt(zmp, zmp[:, m * BC : (m + 1) * BC],
                    wt[:, m * 128 : (m + 1) * 128], blk(OTB))
            mmt(zmp, zmp[0:64, 4 * BC : 5 * BC], wt4[:, 0:64], blk(OTB),
                stop=True)

        def act_zm(s, zmp):
            nc.scalar.activation(st[:, HM0 * BC : (HM0 + 4) * BC],
                                 zmp[:, 0 : 4 * BC], AF.Tanh,
                                 bias=0.0, scale=inv_zm)
            nc.vector.tensor_scalar_max(st[:, HM0 * BC : (HM0 + 4) * BC],
                                        st[:, HM0 * BC : (HM0 + 4) * BC], 0.0)
            nc.scalar.activation(st[0:64, HM4 * BC : HM4 * BC + BC],
                                 zmp[0:64, 4 * BC : 5 * BC], AF.Tanh,
                                 bias=0.0, scale=1.0)
            nc.vector.tensor_scalar_max(st[0:64, HM4 * BC : HM4 * BC + BC],
                                        st[0:64, HM4 * BC : HM4 * BC + BC],
                                        0.0)

        def act_zv(s, zvp):
            nc.scalar.activation(st[:, 0 : 4 * BC], zvp[:, 0 : 4 * BC],
                                 AF.Tanh, bias=0.0, scale=inv_zv)
            nc.vector.tensor_scalar_max(st[:, 0 : 4 * BC],
                                        st[:, 0 : 4 * BC], 0.0)
            nc.scalar.activation(st[0:64, HV4 * BC : HV4 * BC + BC],
                                 zvp[0:64, 4 * BC : 5 * BC], AF.Tanh,
                                 bias=0.0, scale=inv_zv)
            nc.vector.tensor_scalar_max(st[0:64, HV4 * BC : HV4 * BC + BC],
                                        st[0:64, HV4 * BC : HV4 * BC + BC],
                                        0.0)

        def b1_m4(s, zmp, final=False):
            for k in range(5):
                w = wb4[:, k * 128 : (k + 1) * 128]
                r = blk(HM0 + k) if k < 4 else blk(HM4)
                mmt(zmp, zmp[:, 4 * BC : 5 * BC], w, r,
                    stop=(final and k == 4))

        def om_copies(s, zmp, with_omb=True):
            ring = s % OUT_RING
            nc.scalar.activation(outs[64:128, ring * BC : (ring + 1) * BC],
                                 zmp[64:128, 4 * BC : 5 * BC], AF.Copy,
                                 bias=0.0, scale=1.0)
            if with_omb:
                nc.vector.tensor_copy(st[64:128, OMB * BC : OMB * BC + BC],
                                      zmp[64:128, 4 * BC : 5 * BC])

        def b1_m03(s, zmp):
            for k in range(5):
                for m in range(4):
                    w = wb[:, (k * 4 + m) * 128 : (k * 4 + m) * 128 + 128]
                    r = blk(HM0 + k) if k < 4 else blk(HM4)
                    mmt(zmp, zmp[:, m * BC : (m + 1) * BC], w, r)

        def out_dma(s, flush=False):
            """DMA completed output ring halves to DRAM."""
            half = OUT_RING // 2
            if (s + 1) % half == 0:
                lo = ((s + 1 - half) % OUT_RING)
                nc.sync.dma_start(
                    out=d_out[:, (s + 1 - half) * BC : (s + 1) * BC],
                    in_=outs[64:128, lo * BC : (lo + half) * BC],
                )
            elif flush:
                rem = (s + 1) % half
                lo = (s + 1 - rem) % OUT_RING
                nc.sync.dma_start(
                    out=d_out[:, (s + 1 - rem) * BC : (s + 1) * BC],
                    in_=outs[64:128, lo * BC : (lo + rem) * BC],
                )

        def body(T_lo, T_hi, first, last):
            """Emit iterations s in [T_lo, T_hi); first/last flags control
            prologue/epilogue stitching."""
            if first:
                # prologue A(-1): zv(0) full; p(-1) garbage (unread)
                zvp = pp.tile([128, 5 * BC], F32, tag="zv", name="zvpP")
                a_m4(-1, zvp)
                hold_copy(-1)  # hold(0) into both row-64 slots
                a_m03(-1, zvp, range(0, 5), True)
                act_zv(0, zvp)
                zmp = pp.tile([128, 5 * BC], F32, tag="zm", name="zmpP")
                state["zmp"] = zmp
                b1_m4(-1, zmp)   # om(-1) garbage (unread; omb stays 0)
                b1_m03(-1, zmp)
            for s in range(T_lo, T_hi):
                zvp = pp.tile([128, 5 * BC], F32, tag="zv", name=f"zvp{s}")
                a_m4(s, zvp)
                hold_copy(s)
                act_p(s, zvp)
                a_m03(s, zvp, range(0, 3), False)
                zmp = state["zmp"]
                b2(s, zmp)        # completes zm(s)
                act_zm(s, zmp)
                a_m03(s, zvp, range(3, 5), True)
                act_zv(s + 1, zvp)
                zmp2 = pp.tile([128, 5 * BC], F32, tag="zm", name=f"zmp{s}")
                state["zmp"] = zmp2
                b1_m4(s, zmp2)
                om_copies(s, zmp2)
                b1_m03(s, zmp2)
                out_dma(s)
            if last:
                s = T_hi
                zvp = pp.tile([128, 5 * BC], F32, tag="zv", name="zvpE")
                a_m4(s, zvp)
                act_p(s, zvp)
                zmp = state["zmp"]
                b2(s, zmp)
                act_zm(s, zmp)
                zmp2 = pp.tile([128, 5 * BC], F32, tag="zm", name="zmpE")
                b1_m4(s, zmp2, final=True)
                om_copies(s, zmp2, with_omb=False)
                out_dma(s, flush=True)

        if loop_niter is None:
            body(0, T_steps - 1, True, True)
            if debug:
                nc.sync.dma_start(out=d_dbg[:, :], in_=st[:, :])
        else:
            with tc.For_i(0, loop_niter, 1):
                body(0, T_steps - 1, True, True)

        _pp_cm.__exit__(None, None, None)
        for fr in reversed(frees):
            fr()

    if loop_niter is None:
        _thin_updates(nc)
    _split_waits(nc)
    return nc


# scale immediates, set by pack_inputs before build (pow2, input-independent
# only in value; stored as module state so build_nc can bake them as imms)
build_nc._inv_zv = 1.0
build_nc._inv_p = 1.0
build_nc._inv_zm = 1.0


# ----------------------------------------------------------------------------
# Host-side packing
# ----------------------------------------------------------------------------
def _q_e3m4_np(x):
    return np.asarray(x, np.float32).astype(ml_dtypes.float8_e3m4)


def _pow2scale(absmax, cap=15.0):
    return float(2.0 ** np.floor(np.log2(cap / absmax)))


def pack_inputs(inputs, T_steps=T):
    d = {k: np.asarray(v, np.float32) for k, v in inputs.items()}
    W1 = d["h2h_w"].T.copy()              # [576, 576]
    W2 = d["h2hd_w"].T.copy()
    MvT = (d["thal_w"][:, :64] @ d["h2o_w"]).T.copy()   # [576, 64]
    MoT = d["h2od_w"].T.copy()            # [576, 64]
    i2hT = d["i2h_w"].T.copy()            # [128, 512]
    wtmT = d["i2hd_w"][:, :64].T.copy()   # [64, 576]
    thmT = d["thal_w"][:, 65:].T.copy()   # [64, 64]
    holdw_p = d["thal_w"][:, 64].copy()   # [64]
    holdw_m = d["i2hd_w"][:, 64].copy()   # [576]
    bias_v = d["h2h_b"].copy()
    bias_v[:512] += d["i2h_b"]
    bias_p = d["thal_b"] + d["thal_w"][:, :64] @ d["h2o_b"]
    bias_m = d["i2hd_b"] + d["h2hd_b"]
    bias_o = d["h2od_b"].copy()

    S_zv = _pow2scale(max(np.abs(W1).max(), np.abs(i2hT).max(),
                          np.abs(bias_v).max()))
    S_p = _pow2scale(max(np.abs(MvT).max(), np.abs(holdw_p).max(),
                         np.abs(thmT).max(), np.abs(bias_p).max()))
    S_zm = _pow2scale(max(np.abs(W2[:, :512]).max(),
                          np.abs(wtmT[:, :512]).max(),
                          np.abs(holdw_m[:512]).max(),
                          np.abs(bias_m[:512]).max()))
    build_nc._inv_zv = 1.0 / S_zv
    build_nc._inv_p = 1.0 / S_p
    build_nc._inv_zm = 1.0 / S_zm

    CH = [128, 128, 128, 128, 64]

    # A tiles [128, 25*128]: (k,m); m<4 cols = W1*S_zv; m4 = [W1tail | Mv*S_p]
    waf = np.zeros((128, 25 * 128), np.float32)
    for k in range(5):
        kw = CH[k]
        rows = slice(k * 128, k * 128 + kw)
        for m in range(4):
            c = (k * 5 + m) * 128
            waf[0:kw, c : c + 128] = W1[rows, m * 128 : (m + 1) * 128] * S_zv
        c = (k * 5 + 4) * 128
        waf[0:kw, c : c + 64] = W1[rows, 512:576] * S_zv
        waf[0:kw, c + 64 : c + 128] = MvT[rows, :] * S_p
    # k4 hold row (64) + ones/bias row (65)
    c4 = lambda m: (4 * 5 + m) * 128
    for m in range(4):
        waf[65, c4(m) : c4(m) + 128] = bias_v[m * 128 : (m + 1) * 128] * S_zv
    waf[64, c4(4) + 64 : c4(4) + 128] = holdw_p * S_p
    waf[65, c4(4) : c4(4) + 64] = bias_v[512:576] * S_zv
    waf[65, c4(4) + 64 : c4(4) + 128] = bias_p * S_p

    xif = np.zeros((128, 4 * 128), np.float32)
    for m in range(4):
        xif[:, m * 128 : (m + 1) * 128] = i2hT[:, m * 128 : (m + 1) * 128] \
            * S_zv

    wbf = np.zeros((128, 20 * 128), np.float32)
    for k in range(5):
        kw = CH[k]
        rows = slice(k * 128, k * 128 + kw)
        for m in range(4):
            c = (k * 4 + m) * 128
            wbf[0:kw, c : c + 128] = W2[rows, m * 128 : (m + 1) * 128] * S_zm
    cb4 = lambda m: (4 * 4 + m) * 128
    for m in range(4):
        wbf[64, cb4(m) : cb4(m) + 128] = holdw_m[m * 128 : (m + 1) * 128] \
            * S_zm
        wbf[65, cb4(m) : cb4(m) + 128] = bias_m[m * 128 : (m + 1) * 128] \
            * S_zm

    wb4f = np.zeros((128, 5 * 128), np.float32)
    for k in range(5):
        kw = CH[k]
        rows = slice(k * 128, k * 128 + kw)
        wb4f[0:kw, k * 128 : k * 128 + 64] = W2[rows, 512:576]
        wb4f[0:kw, k * 128 + 64 : (k + 1) * 128] = MoT[rows, :]
    wb4f[64, 4 * 128 : 4 * 128 + 64] = holdw_m[512:576]
    wb4f[65, 4 * 128 : 4 * 128 + 64] = bias_m[512:576]
    wb4f[65, 4 * 128 + 64 : 5 * 128] = bias_o

    wtf = np.zeros((128, 4 * 128), np.float32)
    for m in range(4):
        wtf[64:128, m * 128 : (m + 1) * 128] = \
            wtmT[:, m * 128 : (m + 1) * 128] * S_zm
    wt4f = np.zeros((128, 64), np.float32)
    wt4f[64:128, :] = wtmT[:, 512:576]
    thf = np.zeros((128, 64), np.float32)
    thf[64:128, :] = thmT * S_p
    idf = np.zeros((128, 64), np.float32)
    idf[64:128, :] = np.eye(64, dtype=np.float32) * S_zv

    shared = {
        "WA": _q_e3m4_np(waf),
        "XI": _q_e3m4_np(xif),
        "WB": _q_e3m4_np(wbf),
        "WB4": wb4f.astype(ml_dtypes.bfloat16),
        "WT": _q_e3m4_np(wtf),
        "WT4": wt4f.astype(ml_dtypes.bfloat16),
        "TH": _q_e3m4_np(thf),
        "ID64": idf.astype(ml_dtypes.bfloat16),
    }

    data = d["data"]
    in_maps = []
    for c in range(N_CORES):
        sl = slice(c * BC, (c + 1) * BC)
        dc = data[sl, :T_steps, :]  # [BC, T, L]
        xT = np.ascontiguousarray(dc.transpose(2, 1, 0)).reshape(L, -1)
        stf = np.zeros((128, NST * BC), np.float32)
        hv0 = d["h0_v"][sl].T  # [576, BC]
        hm0 = d["h0_m"][sl].T
        for k in range(5):
            kw = CH[k]
            rows = slice(k * 128, k * 128 + kw)
            bv = k if k < 4 else HV4
            bm = HM0 + k if k < 4 else HM4
            stf[0:kw, bv * BC : (bv + 1) * BC] = hv0[rows, :]
            stf[0:kw, bm * BC : (bm + 1) * BC] = hm0[rows, :]
        stf[65, HV4 * BC : (HV4 + 1) * BC] = 1.0  # ones rows (bias)
        stf[65, HM4 * BC : (HM4 + 1) * BC] = 1.0
        m = dict(shared)
        m["xT"] = xT.astype(ml_dtypes.bfloat16)
        m["HD"] = np.ascontiguousarray(xT[127:128, :]).astype(
            ml_dtypes.bfloat16)
        m["ST0"] = stf.astype(ml_dtypes.bfloat16)
        in_maps.append(m)
    return in_maps


def run(inputs, T_steps=T, nc=None):
    in_maps = pack_inputs(inputs, T_steps)  # sets scale imms
    if nc is None:
        nc = build_nc(T_steps)
    res = run_bass_kernel_spmd(nc, in_maps, core_ids=list(range(N_CORES)))
    out = np.zeros((B, T_steps, O), np.float32)
    for c in range(N_CORES):
        o = res.results[c]["OUT"]  # [O, T*BC]
        out[c * BC : (c + 1) * BC] = (
            o.reshape(O, T_steps, BC).transpose(2, 1, 0)
        )
    return out


def kernel(**inputs):
    return run(inputs)


if __name__ == "__main__":
    pass


# revision 86
# speedup vs baseline: 1.4515x; 1.4515x over previous
"""Trainium2 Bass kernel for nn_MilliesRNN (B=256, T=512, L=128, O=64, H=576).

Strategy (v2 — weight-load-bound optimization):
- Data-parallel over batch: 8 cores x 32 sequences; weights replicated;
  states SBUF-resident, transposed ([hidden-chunk, batch]).
- The per-step PE cost is LDWEIGHTS-bound (stationary-tile columns stream
  at ~1 col/cycle; moving N=32 only). v2 cuts weight-load cost by:
  * fp8 (TRN float8e3 = e3m4) stationary tiles for all large weights, with
    bf16 moving states (mixed-dtype matmul, verified bit-exact on HW) ->
    4x faster weight load via FWL. Global power-of-2 scale per output
    group (S_zv, S_p, S_zm); unscale via activation `scale` immediate.
  * Mv (=thal[:, :64] @ h2o_w) merged into W1's m4 column-tiles: the same
    pass over hv computes zv-tail AND p. Mo (=h2od_w^T) merged into W2's
    m4 tiles (bf16 for output accuracy): same pass computes zm-tail AND om.
  * All stationary tiles zero-padded to K=128 (uniform full-row moving
    blocks; ones-row at row 65 carries biases, hold row at 64).
  * Structural 1-step software pipeline: at iter s the merged A-tiles
    compute [zv(s+1) | p(s)] from hv(s); B-tiles compute [zm(s+1) | om(s)]
    from hm(s); wtm joins zm(s) late (after ot(s) exists).
- Per-step activation work minimized (ACT ops cost ~450ns each, nearly
  size-independent): ONE merged tanh per state group over a PSUM tile laid
  out [m4 | m0..3] to match contiguous state blocks; the k4 block's
  hold/ones rows are restored by a 2-row DVE copy after each merged tanh.
  Per step: 3 ACT tanh + 7 small DVE ops + 61 LDW/MM pairs.
- Measured (For_i wall-delta, 8 cores, HW): 3.1-4.6 us/step (burst to
  sustained-throttled) vs baseline 5.5-6.8 us/step, same harness.
  Validated numerics on HW, T=512 full batch: rel L2 = 1.239e-2
  (tolerance 2e-2; numpy sim predicted 1.24e-2; all-bf16 variant 3.2e-3).
"""

import numpy as np
import ml_dtypes

import concourse.bass as bass
import concourse.mybir as mybir
import concourse.tile as tile
from concourse.bass_utils import run_bass_kernel_spmd
from concourse.vector_clock import ScopedClock

# ----------------------------------------------------------------------------
# Workarounds: this walrus build only supports ONE sync-wait per instruction.
# ----------------------------------------------------------------------------
_MAXW = 1


def _patched_drain_and_barrier(self, tick_clock, wait_clock):
    nc = self.nc
    drain_inst = nc.sync.drain()
    wait_clock.add_sem_waits(
        drain_inst.ins, ScopedClock({None: tick_clock.global_clock})
    )
    si = drain_inst.ins.sync_info
    waits = list(si.on_wait) if si is not None else []
    if len(waits) > _MAXW:
        drain_inst.ins.sync_info = mybir.SyncInfo(
            on_wait=waits[:_MAXW], on_update=[]
        )
        rest = waits[_MAXW:]
        for i in range(0, len(rest), _MAXW):
            nop = nc.sync.nop(nofuse=True)
            nop.ins.sync_info = mybir.SyncInfo(
                on_wait=rest[i : i + _MAXW], on_update=[]
            )
    nc.all_engine_barrier()
    assert self.sems is not None
    popped = nc._tile_sem_poison_stack.pop()
    assert popped is self._sem_poison
    nc.clear_and_free_semaphores(list(self.sems.allocated().values()))
    nc.all_engine_barrier()


tile.TileContext._drain_and_barrier = _patched_drain_and_barrier

_wfix_ctr = [0]


def _split_waits(nc, maxw=_MAXW):
    """Move excess sync-waits onto preceding same-engine nops."""
    n_split = 0
    for f in nc.m.functions:
        for b in f.blocks:
            lst = b.instructions
            i = 0
            while i < len(lst):
                inst = lst[i]
                si = getattr(inst, "sync_info", None)
                if si is not None:
                    waits = list(si.on_wait)
                    if len(waits) > maxw:
                        n_split += 1
                        inst.sync_info = mybir.SyncInfo(
                            on_wait=waits[:maxw], on_update=list(si.on_update)
                        )
                        rest = waits[maxw:]
                        for j in range(0, len(rest), maxw):
                            nop = mybir.InstNoOp(
                                name=f"WFIX-{_wfix_ctr[0]}", ins=[], outs=[]
                            )
                            _wfix_ctr[0] += 1
                            nop.engine = inst.engine
                            nop.sync_info = mybir.SyncInfo(
                                on_wait=rest[j : j + maxw], on_update=[]
                            )
                            lst.insert(i, nop)
                            i += 1
                i += 1
    return n_split


def _thin_updates(nc, engines=("EngineType.PE",)):
    """Drop engine-clock sem-incs at positions nobody waits on, and renumber
    the remaining wait thresholds to ranks within the kept set."""
    thresholds = {}
    inc_engines = {}
    _bb_incs = {}
    for f in nc.m.functions:
        for b in f.blocks:
            for inst in b.instructions:
                si = getattr(inst, "sync_info", None)
                if si is None:
                    continue
                for w in si.on_wait:
                    if w.wait_value is not None and w.ant_name:
                        thresholds.setdefault(w.ant_name, set()).add(
                            w.wait_value
                        )
                for u in si.on_update:
                    if u.update_mode == "sem-inc" and u.ant_name:
                        inc_engines.setdefault(u.ant_name, set()).add(
                            str(inst.engine)
                        )
                        k = (u.ant_name, id(b))
                        _bb_incs[k] = _bb_incs.get(k, 0) + 1
    eligible = {
        s for s, engs in inc_engines.items()
        if engs == set(engines) and s in thresholds
    }
    main_bb = {}
    for (s, bid), n in _bb_incs.items():
        if s in eligible and n > main_bb.get(s, (None, 0))[1]:
            main_bb[s] = (bid, n)
    remap = {}
    n_removed = 0
    for f in nc.m.functions:
        for b in f.blocks:
            cum = {}
            kept = {}
            for inst in b.instructions:
                si = getattr(inst, "sync_info", None)
                if si is None or str(inst.engine) not in engines:
                    continue
                keep = []
                changed = False
                for u in si.on_update:
                    if (
                        u.update_mode == "sem-inc"
                        and u.update_value == 1
                        and u.ant_name in eligible
                        and not u.update_reg
                        and main_bb[u.ant_name][0] == id(b)
                    ):
                        s = u.ant_name
                        cum[s] = cum.get(s, 0) + 1
                        if cum[s] in thresholds[s]:
                            kept[s] = kept.get(s, 0) + 1
                            remap.setdefault(s, {})[cum[s]] = kept[s]
                            keep.append(u)
                        else:
                            n_removed += 1
                            changed = True
                    else:
                        keep.append(u)
                if changed:
                    inst.sync_info = mybir.SyncInfo(
                        on_wait=list(si.on_wait), on_update=keep
                    )
    for f in nc.m.functions:
        for b in f.blocks:
            for inst in b.instructions:
                si = getattr(inst, "sync_info", None)
                if si is None:
                    continue
                for w in si.on_wait:
                    m = remap.get(w.ant_name)
                    if m and w.wait_value in m:
                        w.wait_value = m[w.wait_value]
    return n_removed


# ----------------------------------------------------------------------------
# Problem constants
# ----------------------------------------------------------------------------
B, T, L, O, H = 256, 512, 128, 64, 576
N_CORES = 8
BC = B // N_CORES  # 32 batch per core
F32 = mybir.dt.float32
BF16 = mybir.dt.bfloat16
FP8 = mybir.dt.float8e3
AF = mybir.ActivationFunctionType

# megatile column-block indices (each [128, BC])
HV4 = 0      # hv-tail(0:64) + hold(s) row64 + ones row65 (+act garbage)
HV0 = 1      # hv chunks 0..3 at blocks 1..4
HM4 = 5      # hm-tail(0:64) + hold(s+1) row64 + ones row65 (+act garbage)
HM0 = 6      # hm chunks 0..3 at blocks 6..9
OTB = 10     # zeros top, ot at rows 64:128
OMB = 11     # zeros top, om at rows 64:128
NST = 12

OUT_RING = 16  # output staging ring blocks (DMA every OUT_RING//2)


def build_nc(T_steps=T, loop_niter=None, debug=False, pe_only=False,
             thin_loop=False):
    """Per-core SPMD bass kernel, fully unrolled over T_steps.

    loop_niter: timing-only mode — wraps the body in a For_i hardware loop
    (outputs garbage past the first pass; per-step time via wall deltas)."""
    nc = bass.Bass()
    TB = T_steps * BC

    if debug:
        d_dbg = nc.dram_tensor("DBG", [128, NST * BC], BF16,
                               kind="ExternalOutput")
    d_xT = nc.dram_tensor("xT", [L, TB], BF16, kind="ExternalInput")
    d_hd = nc.dram_tensor("HD", [2, TB], BF16, kind="ExternalInput")
    d_wa = nc.dram_tensor("WA", [128, 25 * 128], FP8, kind="ExternalInput")
    d_xi = nc.dram_tensor("XI", [128, 4 * 128], FP8, kind="ExternalInput")
    d_wb = nc.dram_tensor("WB", [128, 20 * 128], FP8, kind="ExternalInput")
    d_wb4 = nc.dram_tensor("WB4", [128, 5 * 128], BF16, kind="ExternalInput")
    d_wt = nc.dram_tensor("WT", [128, 4 * 128], FP8, kind="ExternalInput")
    d_wt4 = nc.dram_tensor("WT4", [128, 64], BF16, kind="ExternalInput")
    d_th = nc.dram_tensor("TH", [128, 64], FP8, kind="ExternalInput")
    d_id = nc.dram_tensor("ID64", [128, 64], BF16, kind="ExternalInput")
    d_st = nc.dram_tensor("ST0", [128, NST * BC], BF16, kind="ExternalInput")
    d_out = nc.dram_tensor("OUT", [O, TB], F32, kind="ExternalOutput")

    frees = []
    with tile.TileContext(nc) as tc:

        def mk(name, shape, dtype):
            t, fr = tc.tile(shape, dtype, name=name)
            frees.append(fr)
            return t

        xTs = mk("xTs", [L, TB], BF16)
        wa = mk("wa", [128, 25 * 128], FP8)
        xi = mk("xi", [128, 4 * 128], FP8)
        wb = mk("wb", [128, 20 * 128], FP8)
        wb4 = mk("wb4", [128, 5 * 128], BF16)
        wt = mk("wt", [128, 4 * 128], FP8)
        wt4 = mk("wt4", [128, 64], BF16)
        th = mk("th", [128, 64], FP8)
        id64 = mk("id64", [128, 64], BF16)
        st = mk("st", [128, NST * BC], BF16)
        outs = mk("outs", [128, OUT_RING * BC], F32)
        # hold row (64) + ones row (65) staged at partition 64/65 (DVE needs
        # 32-aligned partition bases, matching between copy src/dst)
        hrow = mk("hrow", [66, TB], BF16)

        # --- loads (xT split for early start) ---
        NXCH = 8
        xch = TB // NXCH
        for i in range(NXCH):
            nc.sync.dma_start(
                out=xTs[:, i * xch : (i + 1) * xch],
                in_=d_xT[:, i * xch : (i + 1) * xch],
            )
        for tl, dr in ((wa, d_wa), (xi, d_xi), (wb, d_wb), (wb4, d_wb4),
                       (wt, d_wt), (wt4, d_wt4), (th, d_th), (id64, d_id),
                       (st, d_st)):
            nc.sync.dma_start(out=tl, in_=dr[:, :])
        nc.sync.dma_start(out=hrow[64:66, :], in_=d_hd[:, :])

        _pp_cm = tc.tile_pool(name="psum", bufs=3, space="PSUM")
        pp = _pp_cm.__enter__()

        _started = {}

        def mm(out_ap, lhsT, rhs, start, stop):
            nc.tensor.matmul(
                out_ap, lhsT, rhs, start=start, stop=stop,
                skip_group_check=True,
            )

        def mmt(tl, out_ap, lhsT, rhs, stop=False):
            """Matmul with start=True exactly on the first MM into tile tl
            (start clears the whole PSUM bank's has_written bits)."""
            key = id(tl)
            started = _started.get(key, False)
            mm(out_ap, lhsT, rhs, not started, stop)
            _started[key] = True

        def blk(b):
            return st[:, b * BC : (b + 1) * BC]

        # scales are immediates baked at build time via SC? immediates must
        # be python floats at build time -> passed via module-level closure.
        inv_zv = build_nc._inv_zv
        inv_p = build_nc._inv_p
        inv_zm = build_nc._inv_zm

        state = {}

        def a_m4(s, zvp):
            """A(s) m4-part: [zv(s+1)-tail | p(s)] <- 5 wa-k-m4 + thm.
            zvp cols: [m4 | m0..3]."""
            for k in range(5):
                w = wa[:, (k * 5 + 4) * 128 : (k * 5 + 4) * 128 + 128]
                r = blk(HV0 + k) if k < 4 else blk(HV4)
                mmt(zvp, zvp[:, 0:BC], w, r)
            mmt(zvp, zvp[64:128, 0:BC], th[:, 0:64], blk(OMB))

        def hold_copy_a(s):
            """hv4h rows 64:66 <- [hold(s+1), 1.0] — restores hold and ones
            after the merged zv tanh garbaged them."""
            src = hrow[64:66, (s + 1) * BC : (s + 2) * BC]
            nc.vector.tensor_copy(st[64:66, HV4 * BC : HV4 * BC + BC], src)

        def hold_copy_b(s):
            """hm4h rows 64:66 <- [hold(s+1), 1.0] — restores the hold and
            ones rows after the merged zm tanh garbaged them."""
            src = hrow[64:66, (s + 1) * BC : (s + 2) * BC]
            nc.vector.tensor_copy(st[64:66, HM4 * BC : HM4 * BC + BC], src)

        def retanh(dst, src, scale):
            """dst = relu(tanh(src*scale)); both ACT ops (no cross-engine
            sem between them — same queue, program order)."""
            nc.scalar.activation(dst, src, AF.Tanh, bias=0.0, scale=scale)
            nc.scalar.activation(dst, dst, AF.Relu)

        def act_p(s, zvp):
            tanh_dvemax(st[64:128, OTB * BC : OTB * BC + BC],
                        zvp[64:128, 0:BC], inv_p)

        def a_m03(s, zvp, krange, with_xi_id):
            for k in krange:
                for m in range(4):
                    w = wa[:, (k * 5 + m) * 128 : (k * 5 + m) * 128 + 128]
                    r = blk(HV0 + k) if k < 4 else blk(HV4)
                    mmt(zvp, zvp[:, (1 + m) * BC : (2 + m) * BC], w, r)
            if with_xi_id:
                xr = xTs[:, (s + 1) * BC : (s + 2) * BC]
                for m in range(4):
                    mmt(zvp, zvp[:, (1 + m) * BC : (2 + m) * BC],
                        xi[:, m * 128 : (m + 1) * 128], xr)
                mmt(zvp, zvp[0:64, 0:BC], id64[:, 0:64], blk(OTB),
                    stop=True)

        def b2(s, zmp):
            """wtm x ot(s+1) -> completes zm(s+1). zmp cols: [m4|m0..3]."""
            for m in range(4):
                mmt(zmp, zmp[:, (1 + m) * BC : (2 + m) * BC],
                    wt[:, m * 128 : (m + 1) * 128], blk(OTB))
            mmt(zmp, zmp[0:64, 0:BC], wt4[:, 0:64], blk(OTB), stop=True)

        def tanh_dvemax(dst, src, scale):
            """Big blocks: tanh on ACT, max on DVE (engine parallelism)."""
            nc.scalar.activation(dst, src, AF.Tanh, bias=0.0, scale=scale)
            nc.vector.tensor_scalar_max(dst, dst, 0.0)

        def act_zm(s, zmp):
            """ONE tanh+max over [hm4|hm0..3] (st blocks 5..9). The hm4h
            rows 64:128 get tanh-garbage (consumers have zero weights
            there); hold_copy_b restores rows 64:66 afterwards."""
            tanh_dvemax(st[:, HM4 * BC : (HM0 + 4) * BC], zmp[:, 0 : 5 * BC],
                        inv_zm)

        def act_zv(s, zvp):
            """ONE tanh+max over [hv4|hv0..3] (st blocks 0..4); hv4h rows
            64:128 get tanh-garbage, restored by hold_copy_a."""
            tanh_dvemax(st[:, 0 : 5 * BC], zvp[:, 0 : 5 * BC], inv_zv)

        def b1_m4(s, zmp, final=False):
            for k in range(5):
                w = wb4[:, k * 128 : (k + 1) * 128]
                r = blk(HM0 + k) if k < 4 else blk(HM4)
                mmt(zmp, zmp[:, 0:BC], w, r, stop=(final and k == 4))

        def om_copies(s, zmp, with_omb=True):
            ring = s % OUT_RING
            nc.vector.tensor_copy(outs[64:128, ring * BC : (ring + 1) * BC],
                                  zmp[64:128, 0:BC])
            if with_omb:
                nc.vector.tensor_copy(st[64:128, OMB * BC : OMB * BC + BC],
                                      zmp[64:128, 0:BC])

        def b1_m03(s, zmp):
            for k in range(5):
                for m in range(4):
                    w = wb[:, (k * 4 + m) * 128 : (k * 4 + m) * 128 + 128]
                    r = blk(HM0 + k) if k < 4 else blk(HM4)
                    mmt(zmp, zmp[:, (1 + m) * BC : (2 + m) * BC], w, r)

        def out_dma(s, flush=False):
            """DMA completed output ring halves to DRAM."""
            half = OUT_RING // 2
            if (s + 1) % half == 0:
                lo = ((s + 1 - half) % OUT_RING)
                nc.sync.dma_start(
                    out=d_out[:, (s + 1 - half) * BC : (s + 1) * BC],
                    in_=outs[64:128, lo * BC : (lo + half) * BC],
                )
            elif flush:
                rem = (s + 1) % half
                lo = (s + 1 - rem) % OUT_RING
                nc.sync.dma_start(
                    out=d_out[:, (s + 1 - rem) * BC : (s + 1) * BC],
                    in_=outs[64:128, lo * BC : (lo + rem) * BC],
                )

        if pe_only:
            # throughput probe: the same MM stream, no act/copy/DMA deps
            def _noop(*a, **k):
                pass
            act_p = act_zm = act_zv = om_copies = _noop  # noqa: F811
            hold_copy_a = hold_copy_b = out_dma = _noop  # noqa: F811
            # OUT still needs a writer (and outs a writer, f32->f32 DMA):
            nc.vector.memset(outs[:, :], 0.0)
            nc.sync.dma_start(out=d_out[:, 0 : NST * BC],
                              in_=outs[64:128, 0 : NST * BC])

        def body(T_lo, T_hi, first, last):
            """Emit iterations s in [T_lo, T_hi); first/last flags control
            prologue/epilogue stitching."""
            if first:
                # prologue A(-1): zv(0) full; p(-1) garbage (unread)
                zvp = pp.tile([128, 5 * BC], F32, tag="zv", name="zvpP")
                a_m4(-1, zvp)
                hold_copy_b(-1)
                a_m03(-1, zvp, range(0, 5), True)
                act_zv(0, zvp)
                hold_copy_a(-1)  # hold(0); after the garbage-writing act
                zmp = pp.tile([128, 5 * BC], F32, tag="zm", name="zmpP")
                state["zmp"] = zmp
                b1_m4(-1, zmp)   # om(-1) garbage (unread; omb stays 0)
                b1_m03(-1, zmp)
            for s in range(T_lo, T_hi):
                zvp = pp.tile([128, 5 * BC], F32, tag="zv", name=f"zvp{s}")
                a_m4(s, zvp)
                act_p(s, zvp)
                a_m03(s, zvp, range(0, 4), False)
                zmp = state["zmp"]
                b2(s, zmp)        # completes zm(s)
                act_zm(s, zmp)
                hold_copy_b(s)
                a_m03(s, zvp, range(4, 5), True)
                act_zv(s + 1, zvp)
                hold_copy_a(s)
                zmp2 = pp.tile([128, 5 * BC], F32, tag="zm", name=f"zmp{s}")
                state["zmp"] = zmp2
                b1_m4(s, zmp2)
                om_copies(s, zmp2)
                b1_m03(s, zmp2)
                out_dma(s)
            if last:
                s = T_hi
                zvp = pp.tile([128, 5 * BC], F32, tag="zv", name="zvpE")
                a_m4(s, zvp)
                act_p(s, zvp)
                zmp = state["zmp"]
                b2(s, zmp)
                act_zm(s, zmp)
                hold_copy_b(s - 1)  # restore ones row for the final om bias
                zmp2 = pp.tile([128, 5 * BC], F32, tag="zm", name="zmpE")
                b1_m4(s, zmp2, final=True)
                om_copies(s, zmp2, with_omb=False)
                out_dma(s, flush=True)

        if loop_niter is None:
            body(0, T_steps - 1, True, True)
            if debug:
                nc.sync.dma_start(out=d_dbg[:, :], in_=st[:, :])
        else:
            with tc.For_i(0, loop_niter, 1):
                body(0, T_steps - 1, True, True)

        _pp_cm.__exit__(None, None, None)
        for fr in reversed(frees):
            fr()

    if loop_niter is None or thin_loop:
        _thin_updates(nc)
    _split_waits(nc)
    return nc


# scale immediates, set by pack_inputs before build (pow2, input-independent
# only in value; stored as module state so build_nc can bake them as imms)
build_nc._inv_zv = 1.0
build_nc._inv_p = 1.0
build_nc._inv_zm = 1.0


# ----------------------------------------------------------------------------
# Host-side packing
# ----------------------------------------------------------------------------
def _q_e3m4_np(x):
    return np.asarray(x, np.float32).astype(ml_dtypes.float8_e3m4)


def _pow2scale(absmax, cap=15.0):
    return float(2.0 ** np.floor(np.log2(cap / absmax)))


def pack_inputs(inputs, T_steps=T):
    d = {k: np.asarray(v, np.float32) for k, v in inputs.items()}
    W1 = d["h2h_w"].T.copy()              # [576, 576]
    W2 = d["h2hd_w"].T.copy()
    MvT = (d["thal_w"][:, :64] @ d["h2o_w"]).T.copy()   # [576, 64]
    MoT = d["h2od_w"].T.copy()            # [576, 64]
    i2hT = d["i2h_w"].T.copy()            # [128, 512]
    wtmT = d["i2hd_w"][:, :64].T.copy()   # [64, 576]
    thmT = d["thal_w"][:, 65:].T.copy()   # [64, 64]
    holdw_p = d["thal_w"][:, 64].copy()   # [64]
    holdw_m = d["i2hd_w"][:, 64].copy()   # [576]
    bias_v = d["h2h_b"].copy()
    bias_v[:512] += d["i2h_b"]
    bias_p = d["thal_b"] + d["thal_w"][:, :64] @ d["h2o_b"]
    bias_m = d["i2hd_b"] + d["h2hd_b"]
    bias_o = d["h2od_b"].copy()

    S_zv = _pow2scale(max(np.abs(W1).max(), np.abs(i2hT).max(),
                          np.abs(bias_v).max()))
    S_p = _pow2scale(max(np.abs(MvT).max(), np.abs(holdw_p).max(),
                         np.abs(thmT).max(), np.abs(bias_p).max()))
    S_zm = _pow2scale(max(np.abs(W2[:, :512]).max(),
                          np.abs(wtmT[:, :512]).max(),
                          np.abs(holdw_m[:512]).max(),
                          np.abs(bias_m[:512]).max()))
    build_nc._inv_zv = 1.0 / S_zv
    build_nc._inv_p = 1.0 / S_p
    build_nc._inv_zm = 1.0 / S_zm

    CH = [128, 128, 128, 128, 64]

    # A tiles [128, 25*128]: (k,m); m<4 cols = W1*S_zv; m4 = [W1tail | Mv*S_p]
    waf = np.zeros((128, 25 * 128), np.float32)
    for k in range(5):
        kw = CH[k]
        rows = slice(k * 128, k * 128 + kw)
        for m in range(4):
            c = (k * 5 + m) * 128
            waf[0:kw, c : c + 128] = W1[rows, m * 128 : (m + 1) * 128] * S_zv
        c = (k * 5 + 4) * 128
        waf[0:kw, c : c + 64] = W1[rows, 512:576] * S_zv
        waf[0:kw, c + 64 : c + 128] = MvT[rows, :] * S_p
    # k4 hold row (64) + ones/bias row (65)
    c4 = lambda m: (4 * 5 + m) * 128
    for m in range(4):
        waf[65, c4(m) : c4(m) + 128] = bias_v[m * 128 : (m + 1) * 128] * S_zv
    waf[64, c4(4) + 64 : c4(4) + 128] = holdw_p * S_p
    waf[65, c4(4) : c4(4) + 64] = bias_v[512:576] * S_zv
    waf[65, c4(4) + 64 : c4(4) + 128] = bias_p * S_p

    xif = np.zeros((128, 4 * 128), np.float32)
    for m in range(4):
        xif[:, m * 128 : (m + 1) * 128] = i2hT[:, m * 128 : (m + 1) * 128] \
            * S_zv

    wbf = np.zeros((128, 20 * 128), np.float32)
    for k in range(5):
        kw = CH[k]
        rows = slice(k * 128, k * 128 + kw)
        for m in range(4):
            c = (k * 4 + m) * 128
            wbf[0:kw, c : c + 128] = W2[rows, m * 128 : (m + 1) * 128] * S_zm
    cb4 = lambda m: (4 * 4 + m) * 128
    for m in range(4):
        wbf[64, cb4(m) : cb4(m) + 128] = holdw_m[m * 128 : (m + 1) * 128] \
            * S_zm
        wbf[65, cb4(m) : cb4(m) + 128] = bias_m[m * 128 : (m + 1) * 128] \
            * S_zm

    # zm-tail columns are scaled by S_zm too (pow2 — exact in bf16) so ONE
    # merged tanh(x*inv_zm) covers the whole zmp tile; om columns (64:128 of
    # the m4 tiles) stay unscaled (read out raw by DVE copies).
    wb4f = np.zeros((128, 5 * 128), np.float32)
    for k in range(5):
        kw = CH[k]
        rows = slice(k * 128, k * 128 + kw)
        wb4f[0:kw, k * 128 : k * 128 + 64] = W2[rows, 512:576] * S_zm
        wb4f[0:kw, k * 128 + 64 : (k + 1) * 128] = MoT[rows, :]
    wb4f[64, 4 * 128 : 4 * 128 + 64] = holdw_m[512:576] * S_zm
    wb4f[65, 4 * 128 : 4 * 128 + 64] = bias_m[512:576] * S_zm
    wb4f[65, 4 * 128 + 64 : 5 * 128] = bias_o

    wtf = np.zeros((128, 4 * 128), np.float32)
    for m in range(4):
        wtf[64:128, m * 128 : (m + 1) * 128] = \
            wtmT[:, m * 128 : (m + 1) * 128] * S_zm
    wt4f = np.zeros((128, 64), np.float32)
    wt4f[64:128, :] = wtmT[:, 512:576] * S_zm
    thf = np.zeros((128, 64), np.float32)
    thf[64:128, :] = thmT * S_p
    idf = np.zeros((128, 64), np.float32)
    idf[64:128, :] = np.eye(64, dtype=np.float32) * S_zv

    shared = {
        "WA": _q_e3m4_np(waf),
        "XI": _q_e3m4_np(xif),
        "WB": _q_e3m4_np(wbf),
        "WB4": wb4f.astype(ml_dtypes.bfloat16),
        "WT": _q_e3m4_np(wtf),
        "WT4": wt4f.astype(ml_dtypes.bfloat16),
        "TH": _q_e3m4_np(thf),
        "ID64": idf.astype(ml_dtypes.bfloat16),
    }

    data = d["data"]
    in_maps = []
    for c in range(N_CORES):
        sl = slice(c * BC, (c + 1) * BC)
        dc = data[sl, :T_steps, :]  # [BC, T, L]
        xT = np.ascontiguousarray(dc.transpose(2, 1, 0)).reshape(L, -1)
        stf = np.zeros((128, NST * BC), np.float32)
        hv0 = d["h0_v"][sl].T  # [576, BC]
        hm0 = d["h0_m"][sl].T
        for k in range(5):
            kw = CH[k]
            rows = slice(k * 128, k * 128 + kw)
            bv = HV0 + k if k < 4 else HV4
            bm = HM0 + k if k < 4 else HM4
            stf[0:kw, bv * BC : (bv + 1) * BC] = hv0[rows, :]
            stf[0:kw, bm * BC : (bm + 1) * BC] = hm0[rows, :]
        stf[65, HV4 * BC : (HV4 + 1) * BC] = 1.0  # ones rows (bias)
        stf[65, HM4 * BC : (HM4 + 1) * BC] = 1.0
        m = dict(shared)
        m["xT"] = xT.astype(ml_dtypes.bfloat16)
        hd = np.ones((2, xT.shape[1]), np.float32)
        hd[0] = xT[127, :]
        m["HD"] = hd.astype(ml_dtypes.bfloat16)
        m["ST0"] = stf.astype(ml_dtypes.bfloat16)
        in_maps.append(m)
    return in_maps


def run(inputs, T_steps=T, nc=None):
    in_maps = pack_inputs(inputs, T_steps)  # sets scale imms
    if nc is None:
        nc = build_nc(T_steps)
    res = run_bass_kernel_spmd(nc, in_maps, core_ids=list(range(N_CORES)))
    out = np.zeros((B, T_steps, O), np.float32)
    for c in range(N_CORES):
        o = res.results[c]["OUT"]  # [O, T*BC]
        out[c * BC : (c + 1) * BC] = (
            o.reshape(O, T_steps, BC).transpose(2, 1, 0)
        )
    return out


def kernel(**inputs):
    return run(inputs)


if __name__ == "__main__":
    pass
